# revision 9
# baseline (speedup 1.0000x reference)
"""Trainium2 Bass kernel for rank-1-logit self attention.

Reference computation (per batch b):
    q = X @ rot.sum(axis=1)            # [S]   (einsum broadcast collapses the k=3 dim)
    k = X @ ent                        # [S]
    logits[s,t] = q[s] * k[t] / sqrt(d)     (rank-1!)
    out = softmax(logits, axis=-1) @ X

Strategy: data-parallel over batch, one batch per NeuronCore (B=8, 8 cores).
Per core, with rows sorted by q (host-side permutation):
    E_T[t, s] = exp(q[s] * (k[t] - K_half) / sqrt(d))    K = kmin for the low-q
                half, kmax for the high-q half -> exponent <= ~4, never overflows
                and equals exact row-max subtraction up to a per-row constant
                that cancels in the normalization.
    O^T[d, s] = sum_t X[t, d] * E_T[t, s]   (PE matmul, X chunks stationary)
    Z[s]      = sum_t E_T[t, s]             (PE matmul, ones stationary)
    out[s, d] = O^T[d, s] / Z[s]            (PE transpose + per-partition scale)

The exp is a single ScalarE activation per (t-chunk, s-half): the logit
multiply rides the activation's per-partition `scale` operand for free.
"""

import os
import sys

import numpy as np

for _p in ("/opt/trn_rl_repo",):
    if os.path.isdir(_p) and _p not in sys.path:
        sys.path.append(_p)

import concourse.bass as bass
import concourse.mybir as mybir
import concourse.tile as tile
from concourse import bacc
from concourse.bass_utils import run_bass_kernel_spmd
from concourse.masks import make_identity

B, S, D = 8, 2048, 128
TC = S // 128  # t-chunks of 128
NJ = S // 512  # 512-wide s-chunks (PSUM bank width in fp32)
SQRT_D = float(np.sqrt(np.float32(D)))

F32 = mybir.dt.float32
F32R = mybir.dt.float32r  # full-rate fp32 matmul mode (N>=256)


def _build(mm_dtype=F32R):
    nc = bacc.Bacc("TRN2", target_bir_lowering=False, debug=False)
    x = nc.dram_tensor("x", [S, D], mm_dtype, kind="ExternalInput")
    qb = nc.dram_tensor("qb", [128, S], F32, kind="ExternalInput")
    scl = nc.dram_tensor("scl", [128, 2 * TC], F32, kind="ExternalInput")
    o = nc.dram_tensor("o", [S, D], F32, kind="ExternalOutput")
    zscratch = nc.dram_tensor("zs", [1, S], F32)

    EXP = mybir.ActivationFunctionType.Exp

    with tile.TileContext(nc) as tc:
        with (
            tc.tile_pool(name="const", bufs=1) as cpool,
            tc.tile_pool(name="xw", bufs=3) as xpool,
            tc.tile_pool(name="e", bufs=2) as epool,
            tc.tile_pool(name="drain", bufs=1) as dpool,
            tc.tile_pool(name="outp", bufs=3) as opool,
        ):
            qb_sb = cpool.tile([128, S], F32)
            nc.sync.dma_start(qb_sb[:], qb.ap())
            scl_sb = cpool.tile([128, 2 * TC], F32)
            nc.sync.dma_start(scl_sb[:], scl.ap())
            ident = cpool.tile([128, 128], F32)
            make_identity(nc, ident[:])
            ones_f = cpool.tile([128, 1], F32)
            nc.vector.memset(ones_f[:], 1.0)
            ones = cpool.tile([128, 1], mm_dtype)
            nc.scalar.copy(ones[:], ones_f[:])

            ot_sb = dpool.tile([128, S], F32)  # O^T [d, s]
            z_sb = dpool.tile([1, S], F32)

            with (
                tc.tile_pool(name="om", bufs=1, space="PSUM") as ompool,
                tc.tile_pool(name="zp", bufs=1, space="PSUM") as zpool,
            ):
                om_ps = ompool.tile([128, S], F32)  # 4 banks
                z_ps = zpool.tile([1, S], F32)  # 4 banks (partition 0)

                xap = x.ap()
                for c in range(TC):
                    x_sb = xpool.tile([128, D], mm_dtype)
                    nc.sync.dma_start(x_sb[:], xap[c * 128 : (c + 1) * 128, :])
                    e_sb = epool.tile([128, S], mm_dtype)
                    # low-q half (K=kmin) and high-q half (K=kmax)
                    nc.scalar.activation(
                        e_sb[:, 0 : S // 2], qb_sb[:, 0 : S // 2], EXP,
                        bias=0.0, scale=scl_sb[:, c : c + 1],
                    )
                    nc.scalar.activation(
                        e_sb[:, S // 2 : S], qb_sb[:, S // 2 : S], EXP,
                        bias=0.0, scale=scl_sb[:, TC + c : TC + c + 1],
                    )
                    lhs = x_sb[:]
                    onesr = ones[:]
                    for j in range(NJ):
                        rhs = e_sb[:, j * 512 : (j + 1) * 512]
                        nc.tensor.matmul(
                            om_ps[:, j * 512 : (j + 1) * 512], lhs, rhs,
                            start=(c == 0), stop=(c == TC - 1),
                        )
                        nc.tensor.matmul(
                            z_ps[0:1, j * 512 : (j + 1) * 512], onesr, rhs,
                            start=(c == 0), stop=(c == TC - 1),
                        )

                for j in range(NJ):
                    nc.vector.tensor_copy(
                        ot_sb[:, j * 512 : (j + 1) * 512],
                        om_ps[:, j * 512 : (j + 1) * 512],
                    )
                nc.vector.tensor_copy(z_sb[:], z_ps[:])

            # reshape Z [1, 2048] -> [128, 16] via a DRAM bounce so each s-tile's
            # 128 Z values line up with the transposed output tile's partitions
            nc.sync.dma_start(zscratch.ap(), z_sb[:])
            z2 = dpool.tile([128, TC], F32)
            nc.sync.dma_start(z2[:], zscratch.ap().rearrange("a (i p) -> p (a i)", p=128))
            z2r = dpool.tile([128, TC], F32)
            nc.vector.reciprocal(z2r[:], z2[:])

            oap = o.ap()
            with tc.tile_pool(name="tr", bufs=2, space="PSUM") as trpool:
                for i in range(TC):
                    tr_ps = trpool.tile([128, 128], F32)
                    nc.tensor.transpose(tr_ps[:], ot_sb[:, i * 128 : (i + 1) * 128], ident[:])
                    o_sb = opool.tile([128, 128], F32)
                    nc.vector.tensor_scalar_mul(o_sb[:], tr_ps[:], z2r[:, i : i + 1])
                    nc.sync.dma_start(oap[i * 128 : (i + 1) * 128, :], o_sb[:])
    nc.compile()
    return nc


_NC_CACHE = {}


def _get_nc(mm_dtype=F32R):
    key = str(mm_dtype)
    if key not in _NC_CACHE:
        _NC_CACHE[key] = _build(mm_dtype)
    return _NC_CACHE[key]


def kernel(inputs, rotation_params, entangle_params, _trace=False, _mm_dtype=None):
    X = np.ascontiguousarray(np.asarray(inputs, dtype=np.float32))
    rot = np.asarray(rotation_params, dtype=np.float32).reshape(D, 3)
    ent = np.asarray(entangle_params, dtype=np.float32).reshape(D)
    rsum = rot.sum(axis=1)

    in_maps = []
    perms = []
    for b in range(B):
        q = X[b] @ rsum
        k = X[b] @ ent
        perm = np.argsort(q, kind="stable")
        qp = q[perm]
        scl = np.empty((128, 2 * TC), dtype=np.float32)
        scl[:, 0:TC] = ((k - k.min()) / SQRT_D).reshape(TC, 128).T  # low-q half
        scl[:, TC : 2 * TC] = ((k - k.max()) / SQRT_D).reshape(TC, 128).T  # high-q half
        in_maps.append(
            {
                "x": X[b],
                "qb": np.ascontiguousarray(np.broadcast_to(qp, (128, S))),
                "scl": scl,
            }
        )
        perms.append(perm)

    nc = _get_nc(_mm_dtype if _mm_dtype is not None else F32R)
    res = run_bass_kernel_spmd(nc, in_maps, core_ids=list(range(B)), trace=_trace)

    out = np.empty_like(X)
    for b in range(B):
        out[b][perms[b]] = res.results[b]["o"]
    if _trace:
        kernel.last_exec_time_ns = res.exec_time_ns
        kernel.last_results = res
    return out
